# revision 28
# baseline (speedup 1.0000x reference)
"""Trainium2 Bass kernel for nn_CustomAttention (B=8, S=1024, H=1024, NH=16).

Strategy: data-parallel over batch — one batch element per NeuronCore, no
collectives. Host does layout-only prep (transposes / bf16 casts); all FLOPs
run on device.

Per-core dataflow (hsT = hidden_states[b].T in bf16):
  QT[o,s], KT[o,s] = W^T . hsT accumulated per 512-chunk in 1-bank PSUM
  tiles (+bias via per-partition tensor_scalar on the DVE drain, stored bf16).
  V[s,o] = hsT^T . wvT in a [128,1024] PSUM tile shared with the scores
  pool; drained to V' [128, NH*65] bf16 where per head col 64 is a DVE-memset
  ones column, so the ctx matmul's row 64 accumulates the softmax
  denominator. (bv is NOT added here: softmax weights sum to 1, so the V
  bias commutes through attention and is added per-partition at the end.)
  scoresT per (head, s-tile) = KT_h . QT_h -> PSUM [128,1024]; exp on ACT
  (scale=1/sqrt(HD); no max-subtraction — scores ~ N(0,1) in fp32/bf16) ->
  bf16 SBUF. ctx'T accumulated per 512-wide l-chunk in 1-bank PSUM tiles
  ([65,512]), drained to cu [65,1024] f32r. Normalization: DVE reciprocal of
  the den row, broadcast across 64 partitions with a K=1 f32r matmul (ones
  stationary at partition 64), DVE multiply, per-partition add of bv, store
  per head on the ACT queue (stores never block the SP load queue).

Schedule: a software pipeline keeps the PE systolic array >90% busy. Each
head emits 8 scores "slots"; after each slot the ctx matmuls of s-tile j-2
(lag hides ACT latency) plus pump()-driven filler generators run: V'
production (t=0), next o-tile's projection chunks, and deferred
normalizations. PSUM budget: scores/V' pool 2x2 banks + projection 2x1 +
ctx/broadcast 2x1 = 8 banks exactly.

All matmuls bf16 (1 cycle/row at any chunk size in the cost model; fp8
DoubleRow was tried and reverted — e4m3 quantization of exp/V fails the
2e-2 gate on concentrated softmax rows). End-to-end error vs the fp32
reference ~6e-3, dominated by bf16 rounding of q/k/exp/V.
"""
import sys

sys.path.insert(0, "/opt/trn_rl_repo")

import numpy as np
import ml_dtypes
from contextlib import ExitStack

from concourse import bacc, tile, mybir
from concourse.bass_utils import run_bass_kernel_spmd

F32 = mybir.dt.float32
F32R = mybir.dt.float32r
BF16 = mybir.dt.bfloat16
FP8 = mybir.dt.float8e4
AF = mybir.ActivationFunctionType
ALU = mybir.AluOpType
DR = mybir.MatmulPerfMode.DoubleRow

P = 128
HD = 64
N_CORES = 8


def build_program(S, H, NH, num_devices=N_CORES):
    KT = H // P          # contraction tiles (8)
    NT = H // P          # o-tiles (8)
    ST = S // P          # s-tiles (8)
    HPT = P // HD        # heads per o-tile (2)
    NC2 = S // 512       # 512-chunks per row (2)
    assert NH * HD == H and HPT == 2
    SCALE = 1.0 / float(np.sqrt(HD))

    nc = bacc.Bacc(
        "TRN2", target_bir_lowering=False, debug=False, num_devices=num_devices
    )

    hsT = nc.dram_tensor("hsT", [H, S], BF16, kind="ExternalInput")
    # weight streams pre-gathered on host: [128, NT*KT*128], slice per o-tile
    wqS = nc.dram_tensor("wqS", [P, NT * KT * P], BF16, kind="ExternalInput")
    wkS = nc.dram_tensor("wkS", [P, NT * KT * P], BF16, kind="ExternalInput")
    wvT = nc.dram_tensor("wvT", [H, H], BF16, kind="ExternalInput")
    bqT = nc.dram_tensor("bqT", [P, NT], F32, kind="ExternalInput")
    bkT = nc.dram_tensor("bkT", [P, NT], F32, kind="ExternalInput")
    bvT = nc.dram_tensor("bvT", [HD, NH], F32, kind="ExternalInput")
    ones64 = nc.dram_tensor("ones64", [1, HD], F32, kind="ExternalInput")
    outT = nc.dram_tensor("outT", [H, S], F32, kind="ExternalOutput")

    CH = [(a, min(a + 512, S)) for a in range(0, S, 512)]

    with tile.TileContext(nc) as tc, ExitStack() as ctx:
        consts = ctx.enter_context(tc.tile_pool(name="consts", bufs=1))
        hstp = ctx.enter_context(tc.tile_pool(name="hstp", bufs=KT))
        wvp = ctx.enter_context(tc.tile_pool(name="wvp", bufs=KT))
        wstr = ctx.enter_context(tc.tile_pool(name="wstr", bufs=6))
        qtp = ctx.enter_context(tc.tile_pool(name="qtp", bufs=4))
        ktp = ctx.enter_context(tc.tile_pool(name="ktp", bufs=4))
        vvp = ctx.enter_context(tc.tile_pool(name="vvp", bufs=ST))
        exp_pool = ctx.enter_context(tc.tile_pool(name="exp_pool", bufs=14))
        cup = ctx.enter_context(tc.tile_pool(name="cup", bufs=8))
        outp = ctx.enter_context(tc.tile_pool(name="outp", bufs=3))
        # PSUM: big2 (V' accum + scores) 2x2 banks, b1 (proj chunks) 2x1,
        # cxp (ctx chunks + den broadcast) 2x1  => 8 banks exactly
        big2 = ctx.enter_context(tc.tile_pool(name="big2", bufs=2, space="PSUM"))
        b1 = ctx.enter_context(tc.tile_pool(name="b1", bufs=2, space="PSUM"))
        cxp = ctx.enter_context(tc.tile_pool(name="cxp", bufs=2, space="PSUM"))

        # ---- constants ----
        bqT_sb = consts.tile([P, NT], F32, tag="bqT")
        bkT_sb = consts.tile([P, NT], F32, tag="bkT")
        bvT_sb = consts.tile([HD, NH], F32, tag="bvT")
        # ones column for the den-broadcast matmul lives at partition 64 so
        # its base partition matches the den row (row 64 of cu tiles)
        o64_sb = consts.tile([HD + 1, HD], F32R, tag="o64")
        nbias_sb = consts.tile([P, 1], F32, tag="nbias")
        nc.vector.memset(nbias_sb[:], -2.0)

        # ---- input loads: ht interleaved with the first two weight streams
        ht = [None] * KT
        wq_tiles = {}
        wk_tiles = {}

        def load_ht(k):
            t_ = hstp.tile([P, S], BF16, tag="ht", name=f"ht{k}")
            nc.sync.dma_start(out=t_[:], in_=hsT[k * P : (k + 1) * P, :])
            ht[k] = t_

        def load_wstream(t):
            for (wS, pool, store, tag) in (
                (wqS, wstr, wq_tiles, "wq"),
                (wkS, wstr, wk_tiles, "wk"),
            ):
                w_ = pool.tile([P, KT, P], BF16, tag="wstr", name=f"{tag}{t}")
                nc.sync.dma_start(
                    out=w_[:],
                    in_=wS[:, t * KT * P : (t + 1) * KT * P].rearrange(
                        "p (k c) -> p k c", k=KT
                    ),
                )
                store[t] = w_

        load_ht(0)
        load_wstream(0)
        for k in range(1, KT):
            load_ht(k)
        # consts are not needed until the first drain / V' bias — issue them
        # after the startup-critical tiles so they don't delay the first mms
        nc.sync.dma_start(out=bqT_sb[:], in_=bqT[:])
        nc.sync.dma_start(out=bkT_sb[:], in_=bkT[:])
        nc.sync.dma_start(out=bvT_sb[:], in_=bvT[:])
        nc.sync.dma_start(out=o64_sb[HD : HD + 1, :], in_=ones64[:].bitcast(F32R))
        load_wstream(1)
        wv = []
        for k in range(KT):
            w_ = wvp.tile([P, H], BF16, tag="wv", name=f"wv{k}")
            nc.sync.dma_start(out=w_[:], in_=wvT[k * P : (k + 1) * P, :])
            wv.append(w_)

        # ---- filler machinery: generators yielding ~cost_ns after small
        # batches of PE work; pump() drives them between scores tiles so the
        # PE never stalls waiting on ACT (exp) to free a PSUM buffer.
        gens = []

        def pump(budget):
            spent = 0
            while gens and spent < budget:
                try:
                    spent += next(gens[0])
                except StopIteration:
                    gens.pop(0)

        qk = {}

        def gen_proj(t):
            """projection for o-tile t, chunked through 1-bank PSUM."""
            outs = []
            for (wtiles, bias_sb, pool, tag) in (
                (wq_tiles, bqT_sb, qtp, "qt"),
                (wk_tiles, bkT_sb, ktp, "kt"),
            ):
                wt = wtiles.pop(t)
                ot = pool.tile([P, S], BF16, tag=tag, name=f"{tag}{t}")
                for (a, b) in CH:
                    ps = b1.tile([P, 512], F32, tag="b1", name=f"p{tag}{t}_{a}")
                    for k in range(KT):
                        nc.tensor.matmul(
                            ps[:], wt[:, k, :], ht[k][:, a:b],
                            start=(k == 0), stop=(k == KT - 1),
                        )
                        if k == 3:
                            yield 852
                    nc.vector.tensor_scalar_add(
                        ot[:, a:b], ps[:], bias_sb[:, t : t + 1]
                    )
                    yield 852
                outs.append(ot)
            qk[t] = outs

        # ---- V' production (PSUM shared with the scores pool; each m-tile
        # is atomic so its PSUM buffer never spans a scores alloc).
        # V' is stored as fp8e4 s-tile PAIRS [128, 2, NH*65] for the
        # DoubleRow ctx matmul (i-slice = s-tile 2q+i); per head col 64 is a
        # ones column so ctx row 64 accumulates the softmax denominator.
        vv = [None] * ST

        def gen_vprod():
            for m in range(ST):
                ps = big2.tile([P, H], F32, tag="big2", name=f"vps{m}")
                for k in range(KT):
                    lhs = ht[k][:, m * P : (m + 1) * P]
                    for (a, b) in CH:
                        nc.tensor.matmul(
                            ps[:, a:b], lhs, wv[k][:, a:b],
                            start=(k == 0), stop=(k == KT - 1),
                        )
                vv[m] = vvp.tile([P, NH * 65], BF16, tag="vv", name=f"vv{m}")
                vview = vv[m][:].rearrange("p (h e) -> p h e", e=65)
                nc.vector.tensor_copy(
                    vview[:, :, 0:64], ps[:].rearrange("p (h d) -> p h d", d=HD)
                )
                nc.vector.memset(vview[:, :, 64:65], 1.0)
                yield 3413

        def gen_norm(t, cus):
            # reciprocal of the den row (stays on partition 64 — DVE cannot
            # cross partitions), broadcast across 64 partitions via a K=1
            # matmul, then multiply; per-head DMA so the last head's store
            # overlaps the other head's normalization
            ou = outp.tile([P, S], F32, tag="ou", name=f"ou{t}")
            for hh, cu in enumerate(cus):
                rec = cup.tile([65, S], F32R, tag="cu", name=f"rec{t}_{hh}")
                with nc.allow_low_precision(reason="f32r is f32-width"):
                    nc.vector.reciprocal(rec[64:65, :], cu[64:65, :])
                yield 500
                for (a, b) in CH:
                    bc = cxp.tile(
                        [65, 512], F32, tag="cx", name=f"bc{t}_{hh}_{a}"
                    )
                    nc.tensor.matmul(
                        bc[0:64, :], o64_sb[HD : HD + 1, :], rec[64:65, a:b],
                        start=True, stop=True, tile_position=(64, 0),
                    )
                    nc.vector.tensor_mul(
                        ou[hh * HD : (hh + 1) * HD, a:b],
                        cu[0:64, a:b], bc[0:64, :],
                    )
                    nc.vector.tensor_scalar_add(
                        ou[hh * HD : (hh + 1) * HD, a:b],
                        ou[hh * HD : (hh + 1) * HD, a:b],
                        bvT_sb[:, HPT * t + hh : HPT * t + hh + 1],
                    )
                    if t == NT - 1 and hh == 1:
                        nc.scalar.dma_start(
                            out=outT[
                                t * P + hh * HD : t * P + (hh + 1) * HD, a:b
                            ],
                            in_=ou[hh * HD : (hh + 1) * HD, a:b],
                        )
                    yield 400
                if not (t == NT - 1 and hh == 1):
                    nc.scalar.dma_start(
                        out=outT[t * P + hh * HD : t * P + (hh + 1) * HD, :],
                        in_=ou[hh * HD : (hh + 1) * HD, :],
                    )
                yield 100

        def head_slots(head, pump_ns, ctx_live):
            """scores+exp slots for one head; lag-2 inline ctx when live.

            Returns (exs, cu): cu is None unless ctx_live.
            """
            t, hh = divmod(head, HPT)
            qt_t, kt_t = qk[t]
            r0, r1 = hh * HD, (hh + 1) * HD
            h = head
            exs = []
            cx = [None, None]
            cu = (
                cup.tile([65, S], F32R, tag="cu", name=f"cu{head}")
                if ctx_live else None
            )

            def ctx_emit(j2):
                for L, (a, b) in enumerate(CH):
                    if j2 == 0:
                        cx[L] = cxp.tile(
                            [65, 512], F32, tag="cx", name=f"cx{head}_{L}"
                        )
                    nc.tensor.matmul(
                        cx[L][:], vv[j2][:, h * 65 : (h + 1) * 65],
                        exs[j2][:, a:b],
                        start=(j2 == 0), stop=(j2 == ST - 1),
                    )
                    if j2 == ST - 1:
                        nc.vector.tensor_copy(cu[:, a:b], cx[L][:])

            for j in range(ST):
                sc = big2.tile([P, S], F32, tag="big2", name=f"sc{head}_{j}")
                for (a, b) in CH:
                    nc.tensor.matmul(
                        sc[:, a:b],
                        kt_t[r0:r1, j * P : (j + 1) * P],
                        qt_t[r0:r1, a:b],
                        start=True, stop=True,
                        tile_position=(r0, 0),
                    )
                ex = exp_pool.tile([P, S], BF16, tag="ex", name=f"ex{head}_{j}")
                nc.scalar.activation(ex[:], sc[:], AF.Exp, scale=SCALE)
                exs.append(ex)
                if ctx_live and j >= 2:
                    ctx_emit(j - 2)
                pump(pump_ns)
            if ctx_live:
                ctx_emit(ST - 2)
                ctx_emit(ST - 1)
            return exs, cu

        def ctx_block(head, exs):
            t, hh = divmod(head, HPT)
            h = head
            cu = cup.tile([65, S], F32R, tag="cu", name=f"cu{head}")
            for (a, b) in CH:
                cx = cxp.tile([65, 512], F32, tag="cx", name=f"cx{head}_{a}")
                for j in range(ST):
                    nc.tensor.matmul(
                        cx[:], vv[j][:, h * 65 : (h + 1) * 65], exs[j][:, a:b],
                        start=(j == 0), stop=(j == ST - 1),
                    )
                nc.vector.tensor_copy(cu[:, a:b], cx[:])
            return cu

        # ================= schedule =================
        for _ in gen_proj(0):
            pass
        gens.append(gen_vprod())
        gens.append(gen_proj(1))

        # t = 0: head 0 pumps V' hard (ACT has slack early); head 1 runs the
        # steady lag-2 ctx pattern (all vv ready by then); head 0's ctx is a
        # catch-up block afterwards.
        ex0, _ = head_slots(0, 4200, ctx_live=False)
        _, cu1 = head_slots(1, 4200, ctx_live=True)
        pump(10**9)  # finish any leftover V'/proj(1)
        cu0 = ctx_block(0, ex0)
        gens.append(gen_norm(0, (cu0, cu1)))

        for t in range(1, NT):
            if t + 1 < NT:
                load_wstream(t + 1)
                gens.append(gen_proj(t + 1))
            _, cua = head_slots(2 * t, 600, ctx_live=True)
            _, cub = head_slots(2 * t + 1, 600, ctx_live=True)
            gens.append(gen_norm(t, (cua, cub)))
        pump(10**9)

    nc.compile()
    return nc


_CACHE = {}


def _get_program(S, H, NH, num_devices):
    key = (S, H, NH, num_devices)
    if key not in _CACHE:
        _CACHE[key] = build_program(S, H, NH, num_devices)
    return _CACHE[key]


def make_in_maps(hidden_states, Wq, bq, Wk, bk, Wv, bv):
    B, S, H = hidden_states.shape
    NH = H // HD
    NT = H // P
    KT = H // P
    bf16 = ml_dtypes.bfloat16
    # weight stream layout: wS[p, t*KT*128 + k*128 + c] = W[t*128+c, k*128+p]
    wqS = np.ascontiguousarray(
        np.transpose(Wq.reshape(NT, P, KT, P), (3, 0, 2, 1)).reshape(P, -1)
    ).astype(bf16)
    wkS = np.ascontiguousarray(
        np.transpose(Wk.reshape(NT, P, KT, P), (3, 0, 2, 1)).reshape(P, -1)
    ).astype(bf16)
    wvT = np.ascontiguousarray(Wv.T).astype(bf16)
    bqT = np.ascontiguousarray(bq.reshape(NT, P).T.astype(np.float32))
    bkT = np.ascontiguousarray(bk.reshape(NT, P).T.astype(np.float32))
    bvT = np.ascontiguousarray(bv.astype(np.float32).reshape(NH, HD).T)
    ones64 = np.ones((1, HD), np.float32)
    in_maps = []
    for b in range(B):
        in_maps.append(
            {
                "hsT": np.ascontiguousarray(hidden_states[b].T).astype(bf16),
                "wqS": wqS,
                "wkS": wkS,
                "wvT": wvT,
                "bqT": bqT,
                "bkT": bkT,
                "bvT": bvT,
                "ones64": ones64,
            }
        )
    return in_maps


def kernel(hidden_states, Wq, bq, Wk, bk, Wv, bv):
    hidden_states = np.asarray(hidden_states, dtype=np.float32)
    Wq = np.asarray(Wq, dtype=np.float32)
    bq = np.asarray(bq, dtype=np.float32)
    Wk = np.asarray(Wk, dtype=np.float32)
    bk = np.asarray(bk, dtype=np.float32)
    Wv = np.asarray(Wv, dtype=np.float32)
    bv = np.asarray(bv, dtype=np.float32)

    B, S, H = hidden_states.shape
    NH = H // HD
    assert B == N_CORES, "one batch element per core"

    nc = _get_program(S, H, NH, N_CORES)
    in_maps = make_in_maps(hidden_states, Wq, bq, Wk, bk, Wv, bv)
    res = run_bass_kernel_spmd(nc, in_maps, core_ids=list(range(N_CORES)))
    out = np.empty((B, S, H), np.float32)
    for b in range(B):
        out[b] = res.results[b]["outT"].T
    return out


if __name__ == "__main__":
    build_program(1024, 1024, 16)
    print("build ok")


# revision 31
# speedup vs baseline: 1.0013x; 1.0013x over previous
"""Trainium2 Bass kernel for nn_CustomAttention (B=8, S=1024, H=1024, NH=16).

Strategy: data-parallel over batch — one batch element per NeuronCore, no
collectives. Host does layout-only prep (transposes / bf16 casts); all FLOPs
run on device.

Per-core dataflow (hsT = hidden_states[b].T in bf16):
  QT[o,s], KT[o,s] = W^T . hsT accumulated per 512-chunk in 1-bank PSUM
  tiles (+bias via per-partition tensor_scalar on the DVE drain, stored bf16).
  V[s,o] = hsT^T . wvT in a [128,1024] PSUM tile shared with the scores
  pool; drained to V' [128, NH*65] bf16 where per head col 64 is a DVE-memset
  ones column, so the ctx matmul's row 64 accumulates the softmax
  denominator. (bv is NOT added here: softmax weights sum to 1, so the V
  bias commutes through attention and is added per-partition at the end.)
  scoresT per (head, s-tile) = KT_h . QT_h -> PSUM [128,1024]; exp on ACT
  (scale=1/sqrt(HD); no max-subtraction — scores ~ N(0,1) in fp32/bf16) ->
  bf16 SBUF. ctx'T accumulated per 512-wide l-chunk in 1-bank PSUM tiles
  ([65,512]), drained to cu [65,1024] f32r. Normalization: DVE reciprocal of
  the den row, broadcast across 64 partitions with a K=1 f32r matmul (ones
  stationary at partition 64), DVE multiply, per-partition add of bv, store
  per head on the ACT queue (stores never block the SP load queue).

Schedule: a software pipeline keeps the PE systolic array >90% busy. Each
head emits 8 scores "slots"; after each slot the ctx matmuls of s-tile j-2
(lag hides ACT latency) plus pump()-driven filler generators run: V'
production (t=0), next o-tile's projection chunks, and deferred
normalizations. PSUM budget: scores/V' pool 2x2 banks + projection 2x1 +
ctx/broadcast 2x1 = 8 banks exactly.

All matmuls bf16 (1 cycle/row at any chunk size in the cost model; fp8
DoubleRow was tried and reverted — e4m3 quantization of exp/V fails the
2e-2 gate on concentrated softmax rows). End-to-end error vs the fp32
reference ~6e-3, dominated by bf16 rounding of q/k/exp/V.
"""
import sys

sys.path.insert(0, "/opt/trn_rl_repo")

import numpy as np
import ml_dtypes
from contextlib import ExitStack

from concourse import bacc, tile, mybir
from concourse.bass_utils import run_bass_kernel_spmd

F32 = mybir.dt.float32
F32R = mybir.dt.float32r
BF16 = mybir.dt.bfloat16
FP8 = mybir.dt.float8e4
AF = mybir.ActivationFunctionType
ALU = mybir.AluOpType
DR = mybir.MatmulPerfMode.DoubleRow

P = 128
HD = 64
N_CORES = 8


def build_program(S, H, NH, num_devices=N_CORES):
    KT = H // P          # contraction tiles (8)
    NT = H // P          # o-tiles (8)
    ST = S // P          # s-tiles (8)
    HPT = P // HD        # heads per o-tile (2)
    NC2 = S // 512       # 512-chunks per row (2)
    assert NH * HD == H and HPT == 2
    SCALE = 1.0 / float(np.sqrt(HD))

    nc = bacc.Bacc(
        "TRN2", target_bir_lowering=False, debug=False, num_devices=num_devices
    )

    hsT = nc.dram_tensor("hsT", [H, S], BF16, kind="ExternalInput")
    # weight streams pre-gathered on host: [128, NT*KT*128], slice per o-tile
    wqS = nc.dram_tensor("wqS", [P, NT * KT * P], BF16, kind="ExternalInput")
    wkS = nc.dram_tensor("wkS", [P, NT * KT * P], BF16, kind="ExternalInput")
    wvT = nc.dram_tensor("wvT", [H, H], BF16, kind="ExternalInput")
    bqT = nc.dram_tensor("bqT", [P, NT], F32, kind="ExternalInput")
    bkT = nc.dram_tensor("bkT", [P, NT], F32, kind="ExternalInput")
    bvT = nc.dram_tensor("bvT", [HD, NH], F32, kind="ExternalInput")
    ones64 = nc.dram_tensor("ones64", [1, HD], F32, kind="ExternalInput")
    outT = nc.dram_tensor("outT", [H, S], F32, kind="ExternalOutput")

    CH = [(a, min(a + 512, S)) for a in range(0, S, 512)]

    with tile.TileContext(nc) as tc, ExitStack() as ctx:
        consts = ctx.enter_context(tc.tile_pool(name="consts", bufs=1))
        hstp = ctx.enter_context(tc.tile_pool(name="hstp", bufs=KT))
        wvp = ctx.enter_context(tc.tile_pool(name="wvp", bufs=KT))
        wstr = ctx.enter_context(tc.tile_pool(name="wstr", bufs=6))
        qtp = ctx.enter_context(tc.tile_pool(name="qtp", bufs=4))
        ktp = ctx.enter_context(tc.tile_pool(name="ktp", bufs=4))
        vvp = ctx.enter_context(tc.tile_pool(name="vvp", bufs=ST))
        exp_pool = ctx.enter_context(tc.tile_pool(name="exp_pool", bufs=14))
        cup = ctx.enter_context(tc.tile_pool(name="cup", bufs=8))
        outp = ctx.enter_context(tc.tile_pool(name="outp", bufs=3))
        # PSUM: big2 (V' accum + scores) 2x2 banks, b1 (proj chunks) 2x1,
        # cxp (ctx chunks + den broadcast) 2x1  => 8 banks exactly
        big2 = ctx.enter_context(tc.tile_pool(name="big2", bufs=2, space="PSUM"))
        b1 = ctx.enter_context(tc.tile_pool(name="b1", bufs=2, space="PSUM"))
        cxp = ctx.enter_context(tc.tile_pool(name="cxp", bufs=2, space="PSUM"))

        # ---- constants ----
        bqT_sb = consts.tile([P, NT], F32, tag="bqT")
        bkT_sb = consts.tile([P, NT], F32, tag="bkT")
        bvT_sb = consts.tile([HD, NH], F32, tag="bvT")
        # ones column for the den-broadcast matmul lives at partition 64 so
        # its base partition matches the den row (row 64 of cu tiles)
        o64_sb = consts.tile([HD + 1, HD], F32R, tag="o64")
        nbias_sb = consts.tile([P, 1], F32, tag="nbias")
        nc.vector.memset(nbias_sb[:], -2.0)

        # ---- input loads: ht interleaved with the first two weight streams
        ht = [None] * KT
        wq_tiles = {}
        wk_tiles = {}

        def load_ht(k):
            t_ = hstp.tile([P, S], BF16, tag="ht", name=f"ht{k}")
            nc.sync.dma_start(out=t_[:], in_=hsT[k * P : (k + 1) * P, :])
            ht[k] = t_

        def load_wstream(t):
            for (wS, pool, store, tag) in (
                (wqS, wstr, wq_tiles, "wq"),
                (wkS, wstr, wk_tiles, "wk"),
            ):
                w_ = pool.tile([P, KT, P], BF16, tag="wstr", name=f"{tag}{t}")
                nc.sync.dma_start(
                    out=w_[:],
                    in_=wS[:, t * KT * P : (t + 1) * KT * P].rearrange(
                        "p (k c) -> p k c", k=KT
                    ),
                )
                store[t] = w_

        load_ht(0)
        load_wstream(0)
        for k in range(1, KT):
            load_ht(k)
        # consts are not needed until the first drain / V' bias — issue them
        # after the startup-critical tiles so they don't delay the first mms
        nc.sync.dma_start(out=bqT_sb[:], in_=bqT[:])
        nc.sync.dma_start(out=bkT_sb[:], in_=bkT[:])
        nc.sync.dma_start(out=bvT_sb[:], in_=bvT[:])
        nc.sync.dma_start(out=o64_sb[HD : HD + 1, :], in_=ones64[:].bitcast(F32R))
        load_wstream(1)
        wv = []
        for k in range(KT):
            w_ = wvp.tile([P, H], BF16, tag="wv", name=f"wv{k}")
            nc.sync.dma_start(out=w_[:], in_=wvT[k * P : (k + 1) * P, :])
            wv.append(w_)

        # ---- filler machinery: generators yielding ~cost_ns after small
        # batches of PE work; pump() drives them between scores tiles so the
        # PE never stalls waiting on ACT (exp) to free a PSUM buffer.
        gens = []

        def pump(budget):
            spent = 0
            while gens and spent < budget:
                try:
                    spent += next(gens[0])
                except StopIteration:
                    gens.pop(0)

        qk = {}

        def gen_proj(t):
            """projection for o-tile t, chunked through 1-bank PSUM."""
            outs = []
            for (wtiles, bias_sb, pool, tag) in (
                (wq_tiles, bqT_sb, qtp, "qt"),
                (wk_tiles, bkT_sb, ktp, "kt"),
            ):
                wt = wtiles.pop(t)
                ot = pool.tile([P, S], BF16, tag=tag, name=f"{tag}{t}")
                for (a, b) in CH:
                    ps = b1.tile([P, 512], F32, tag="b1", name=f"p{tag}{t}_{a}")
                    for k in range(KT):
                        nc.tensor.matmul(
                            ps[:], wt[:, k, :], ht[k][:, a:b],
                            start=(k == 0), stop=(k == KT - 1),
                        )
                        if k == 3:
                            yield 852
                    nc.vector.tensor_scalar_add(
                        ot[:, a:b], ps[:], bias_sb[:, t : t + 1]
                    )
                    yield 852
                outs.append(ot)
            qk[t] = outs

        # ---- V' production (PSUM shared with the scores pool; each m-tile
        # is atomic so its PSUM buffer never spans a scores alloc).
        # V' is stored as fp8e4 s-tile PAIRS [128, 2, NH*65] for the
        # DoubleRow ctx matmul (i-slice = s-tile 2q+i); per head col 64 is a
        # ones column so ctx row 64 accumulates the softmax denominator.
        vv = [None] * ST

        def gen_vprod():
            for m in range(ST):
                ps = big2.tile([P, H], F32, tag="big2", name=f"vps{m}")
                for k in range(KT):
                    lhs = ht[k][:, m * P : (m + 1) * P]
                    for (a, b) in CH:
                        nc.tensor.matmul(
                            ps[:, a:b], lhs, wv[k][:, a:b],
                            start=(k == 0), stop=(k == KT - 1),
                        )
                vv[m] = vvp.tile([P, NH * 65], BF16, tag="vv", name=f"vv{m}")
                vview = vv[m][:].rearrange("p (h e) -> p h e", e=65)
                nc.vector.tensor_copy(
                    vview[:, :, 0:64], ps[:].rearrange("p (h d) -> p h d", d=HD)
                )
                nc.vector.memset(vview[:, :, 64:65], 1.0)
                yield 3413

        ou_hold = {}

        def gen_norm(t, hh, cu):
            # reciprocal of the den row (stays on partition 64 — DVE cannot
            # cross partitions), broadcast across 64 partitions via a K=1
            # matmul, then multiply; per-head DMA so the last head's store
            # overlaps the other head's normalization
            if t not in ou_hold:
                ou_hold[t] = outp.tile([P, S], F32, tag="ou", name=f"ou{t}")
            ou = ou_hold[t]
            if True:
                rec = cup.tile([65, S], F32R, tag="cu", name=f"rec{t}_{hh}")
                with nc.allow_low_precision(reason="f32r is f32-width"):
                    nc.vector.reciprocal(rec[64:65, :], cu[64:65, :])
                yield 500
                for (a, b) in CH:
                    bc = cxp.tile(
                        [65, 512], F32, tag="cx", name=f"bc{t}_{hh}_{a}"
                    )
                    nc.tensor.matmul(
                        bc[0:64, :], o64_sb[HD : HD + 1, :], rec[64:65, a:b],
                        start=True, stop=True, tile_position=(64, 0),
                    )
                    nc.vector.tensor_mul(
                        ou[hh * HD : (hh + 1) * HD, a:b],
                        cu[0:64, a:b], bc[0:64, :],
                    )
                    nc.vector.tensor_scalar_add(
                        ou[hh * HD : (hh + 1) * HD, a:b],
                        ou[hh * HD : (hh + 1) * HD, a:b],
                        bvT_sb[:, HPT * t + hh : HPT * t + hh + 1],
                    )
                    if t == NT - 1 and hh == 1:
                        nc.scalar.dma_start(
                            out=outT[
                                t * P + hh * HD : t * P + (hh + 1) * HD, a:b
                            ],
                            in_=ou[hh * HD : (hh + 1) * HD, a:b],
                        )
                    yield 400
                if not (t == NT - 1 and hh == 1):
                    nc.scalar.dma_start(
                        out=outT[t * P + hh * HD : t * P + (hh + 1) * HD, :],
                        in_=ou[hh * HD : (hh + 1) * HD, :],
                    )
                yield 100

        def head_slots(head, pump_ns, ctx_live):
            """scores+exp slots for one head; lag-2 inline ctx when live.

            Returns (exs, cu): cu is None unless ctx_live.
            """
            t, hh = divmod(head, HPT)
            qt_t, kt_t = qk[t]
            r0, r1 = hh * HD, (hh + 1) * HD
            h = head
            exs = []
            cx = [None, None]
            cu = (
                cup.tile([65, S], F32R, tag="cu", name=f"cu{head}")
                if ctx_live else None
            )

            def ctx_emit(j2):
                for L, (a, b) in enumerate(CH):
                    if j2 == 0:
                        cx[L] = cxp.tile(
                            [65, 512], F32, tag="cx", name=f"cx{head}_{L}"
                        )
                    nc.tensor.matmul(
                        cx[L][:], vv[j2][:, h * 65 : (h + 1) * 65],
                        exs[j2][:, a:b],
                        start=(j2 == 0), stop=(j2 == ST - 1),
                    )
                    if j2 == ST - 1:
                        nc.vector.tensor_copy(cu[:, a:b], cx[L][:])

            for j in range(ST):
                sc = big2.tile([P, S], F32, tag="big2", name=f"sc{head}_{j}")
                for (a, b) in CH:
                    nc.tensor.matmul(
                        sc[:, a:b],
                        kt_t[r0:r1, j * P : (j + 1) * P],
                        qt_t[r0:r1, a:b],
                        start=True, stop=True,
                        tile_position=(r0, 0),
                    )
                ex = exp_pool.tile([P, S], BF16, tag="ex", name=f"ex{head}_{j}")
                nc.scalar.activation(ex[:], sc[:], AF.Exp, scale=SCALE)
                exs.append(ex)
                if ctx_live and j >= 2:
                    ctx_emit(j - 2)
                pump(pump_ns)
            if ctx_live:
                ctx_emit(ST - 2)
                ctx_emit(ST - 1)
            return exs, cu

        def ctx_block(head, exs):
            t, hh = divmod(head, HPT)
            h = head
            cu = cup.tile([65, S], F32R, tag="cu", name=f"cu{head}")
            for (a, b) in CH:
                cx = cxp.tile([65, 512], F32, tag="cx", name=f"cx{head}_{a}")
                for j in range(ST):
                    nc.tensor.matmul(
                        cx[:], vv[j][:, h * 65 : (h + 1) * 65], exs[j][:, a:b],
                        start=(j == 0), stop=(j == ST - 1),
                    )
                nc.vector.tensor_copy(cu[:, a:b], cx[:])
            return cu

        # ================= schedule =================
        for _ in gen_proj(0):
            pass
        gens.append(gen_vprod())
        gens.append(gen_proj(1))

        # t = 0: head 0 pumps V' hard (ACT has slack early); head 1 runs the
        # steady lag-2 ctx pattern (all vv ready by then); head 0's ctx is a
        # catch-up block afterwards.
        ex0, _ = head_slots(0, 4200, ctx_live=False)
        _, cu1 = head_slots(1, 4200, ctx_live=True)
        pump(10**9)  # finish any leftover V'/proj(1)
        cu0 = ctx_block(0, ex0)
        gens.append(gen_norm(0, 1, cu1))
        gens.append(gen_norm(0, 0, cu0))

        for t in range(1, NT):
            if t + 1 < NT:
                load_wstream(t + 1)
                gens.append(gen_proj(t + 1))
            _, cua = head_slots(2 * t, 420, ctx_live=True)
            gens.append(gen_norm(t, 0, cua))
            _, cub = head_slots(2 * t + 1, 420, ctx_live=True)
            gens.append(gen_norm(t, 1, cub))
        pump(10**9)

    nc.compile()
    return nc


_CACHE = {}


def _get_program(S, H, NH, num_devices):
    key = (S, H, NH, num_devices)
    if key not in _CACHE:
        _CACHE[key] = build_program(S, H, NH, num_devices)
    return _CACHE[key]


def make_in_maps(hidden_states, Wq, bq, Wk, bk, Wv, bv):
    B, S, H = hidden_states.shape
    NH = H // HD
    NT = H // P
    KT = H // P
    bf16 = ml_dtypes.bfloat16
    # weight stream layout: wS[p, t*KT*128 + k*128 + c] = W[t*128+c, k*128+p]
    wqS = np.ascontiguousarray(
        np.transpose(Wq.reshape(NT, P, KT, P), (3, 0, 2, 1)).reshape(P, -1)
    ).astype(bf16)
    wkS = np.ascontiguousarray(
        np.transpose(Wk.reshape(NT, P, KT, P), (3, 0, 2, 1)).reshape(P, -1)
    ).astype(bf16)
    wvT = np.ascontiguousarray(Wv.T).astype(bf16)
    bqT = np.ascontiguousarray(bq.reshape(NT, P).T.astype(np.float32))
    bkT = np.ascontiguousarray(bk.reshape(NT, P).T.astype(np.float32))
    bvT = np.ascontiguousarray(bv.astype(np.float32).reshape(NH, HD).T)
    ones64 = np.ones((1, HD), np.float32)
    in_maps = []
    for b in range(B):
        in_maps.append(
            {
                "hsT": np.ascontiguousarray(hidden_states[b].T).astype(bf16),
                "wqS": wqS,
                "wkS": wkS,
                "wvT": wvT,
                "bqT": bqT,
                "bkT": bkT,
                "bvT": bvT,
                "ones64": ones64,
            }
        )
    return in_maps


def kernel(hidden_states, Wq, bq, Wk, bk, Wv, bv):
    hidden_states = np.asarray(hidden_states, dtype=np.float32)
    Wq = np.asarray(Wq, dtype=np.float32)
    bq = np.asarray(bq, dtype=np.float32)
    Wk = np.asarray(Wk, dtype=np.float32)
    bk = np.asarray(bk, dtype=np.float32)
    Wv = np.asarray(Wv, dtype=np.float32)
    bv = np.asarray(bv, dtype=np.float32)

    B, S, H = hidden_states.shape
    NH = H // HD
    assert B == N_CORES, "one batch element per core"

    nc = _get_program(S, H, NH, N_CORES)
    in_maps = make_in_maps(hidden_states, Wq, bq, Wk, bk, Wv, bv)
    res = run_bass_kernel_spmd(nc, in_maps, core_ids=list(range(N_CORES)))
    out = np.empty((B, S, H), np.float32)
    for b in range(B):
        out[b] = res.results[b]["outT"].T
    return out


if __name__ == "__main__":
    build_program(1024, 1024, 16)
    print("build ok")
